# revision 31
# baseline (speedup 1.0000x reference)
"""Trainium2 Bass kernel for nn_AssociativeMemory (Hopfield recall).

Computes state <- tanh(W @ state) for 10 iterations, W: [8192, 8192] f32.

Strategy (8 NeuronCores, SPMD):
  - Row-shard W: core r owns rows [r*1024, (r+1)*1024).
  - Host-side, W_shard is transposed to k-major and split into an fp16
    hi/lo pair, giving ~22 effective mantissa bits while streaming
    through the PE at 1 col/cycle (true fp32 matmul is 4x slower).
    The state vector is likewise hi/lo split on-device each iteration.
    Final accuracy ~1e-3 rel err -- at the intrinsic fp32
    summation-order noise floor of this chaotic recurrence.
  - Scale design: Wl = fp16((W - Wh) * 2^6); A-pass stationary
    [sh, sl*2^12], B-pass stationary [sh*2^-6, sl*2^6].  Then
    A0/B0 share scale 1 and A1/B1 share scale 2^12, so ALL 512
    matmuls/iter accumulate into ONE [2, 1024] PSUM tile:
      row0 = sh.(Wh + Wl_true),  row1 = 2^12 * sl.(Wh + Wl_true)
      y = row0 + row1 / 2^12.
    All scale factors are powers of two (exact) and keep every fp16
    value in the normal range (no subnormal-flush hazard).
  - Matvec on PE: stationary M=2, moving = W chunks [128k x 512i].
    Contraction k is chunked as k = p*64 + c so every DMA (weights,
    state reload) is contiguous -- no transposes anywhere.
  - Most of W stays resident in SBUF across all 10 iterations; the
    overflow streams from HBM each iteration behind the PE.
  - Between iterations: combine rows (one cross-partition DMA move),
    AllGather the 4KB pre-activation, tanh + splits at [128, 64].
"""

import numpy as np

import concourse.bass as bass
import concourse.mybir as mybir
import concourse.tile as tile
from concourse import bacc
from concourse.bass_utils import run_bass_kernel_spmd

P = 8192
N_CORES = 8
ROWS = P // N_CORES          # 1024 output rows per core
NPART = 128                  # SBUF partitions / PE contraction size
CHUNKS = P // NPART          # 64 contraction chunks; global k = p*64 + c
HALF = 512                   # PE moving free-dim per matmul (PSUM bank)
ITERATIONS = 10
SL_SCALE = 4096.0            # 2^12: lo-part of s for the A (Wh) pass
WL_SCALE = 64.0              # 2^6:  lo-part of W storage scale
EPS = 1.0 / SL_SCALE

# W chunk-units resident in SBUF: 64 Wh chunks + RESIDENT_WL Wl chunks
# (2KB/partition each).  The rest of Wl streams from HBM every iteration.
RESIDENT_WL = 20

# Dependency-free PE matmuls issued between iterations: keep the HAM
# activity monitor warm through the inter-iteration gap (~19us) so real
# bursts start at 2.4GHz instead of re-warming from 1.2GHz.
WARM_MMS = 80

_CACHED = {}


def _build_nc():
    # Bacc (not raw Bass): its generate_event_semaphores pass splits
    # multi-wait instructions (HW allows 1 wait/inst) via event semaphores.
    nc = bacc.Bacc(None, target_bir_lowering=False)
    f16 = mybir.dt.float16
    f32 = mybir.dt.float32

    xin = nc.dram_tensor("xin", [P], f32, kind="ExternalInput")
    wh = nc.dram_tensor("wh", [NPART, CHUNKS, ROWS], f16, kind="ExternalInput")
    wl = nc.dram_tensor("wl", [NPART, CHUNKS, ROWS], f16, kind="ExternalInput")
    out = nc.dram_tensor("out", [P], f32, kind="ExternalOutput")

    with tile.TileContext(nc) as tc:
        with (
            tc.tile_pool(name="wres", bufs=1) as wres,
            tc.tile_pool(name="stream", bufs=6) as stream,
            tc.tile_pool(name="state", bufs=1) as state,
            tc.tile_pool(name="tmp", bufs=1) as tmp,
            tc.tile_pool(name="psum", bufs=2, space="PSUM") as psum,
            tc.tile_pool(name="dram", bufs=1, space="DRAM") as dram,
        ):
            # ---- resident weights: tiles allocated here, but the loads are
            # issued just-in-time inside iteration 0 so the PE pipelines
            # behind the HBM stream instead of waiting for the full load.
            wh_sb = wres.tile([NPART, CHUNKS, ROWS], f16)
            wl_sb = wres.tile([NPART, RESIDENT_WL, ROWS], f16)

            # ---- state stationaries: [p, col, c] fp16, k = p*64 + c ----
            # s_a = [sh, sl*2^12] for the Wh pass
            # s_b = [sh*2^-6, sl*2^6] for the Wl pass
            s_a = state.tile([NPART, 2, CHUNKS], f16)
            s_b = state.tile([NPART, 2, CHUNKS], f16)
            d_sb = state.tile([NPART, CHUNKS], f32)

            def split_state(src_f32):
                """hi/lo split of state tile [128, 64] into s_a / s_b."""
                nc.vector.tensor_copy(s_a[:, 0, :], src_f32[:])
                nc.vector.tensor_tensor(
                    d_sb[:], src_f32[:], s_a[:, 0, :], mybir.AluOpType.subtract
                )
                nc.vector.tensor_scalar_mul(s_a[:, 1, :], d_sb[:], SL_SCALE)
                nc.vector.tensor_scalar_mul(s_b[:, 0, :], s_a[:, 0, :], 1.0 / WL_SCALE)
                nc.vector.tensor_scalar_mul(s_b[:, 1, :], d_sb[:], WL_SCALE)

            # initial split of x (no tanh on iteration-1 input)
            x_sb = state.tile([NPART, CHUNKS], f32)
            nc.sync.dma_start(x_sb[:], xin.rearrange("(p c) -> p c", p=NPART))
            split_state(x_sb)

            for it in range(ITERATIONS):
                acc = psum.tile([2, ROWS], f32, tag="acc")

                # 512 matmuls, all accumulating into `acc`:
                # per chunk: A-h0, A-h1 (Wh) then B-h0, B-h1 (Wl)
                for c in range(CHUNKS):
                    if it == 0:
                        # alternate HWDGE/SWDGE so more SDMA engines pull
                        eng = nc.sync if c % 2 == 0 else nc.gpsimd
                        eng.dma_start(wh_sb[:, c, :], wh[:, c, :])
                        if c < RESIDENT_WL:
                            eng.dma_start(wl_sb[:, c, :], wl[:, c, :])
                    if c < RESIDENT_WL:
                        wl_rhs = wl_sb[:, c, :]
                    elif (c - RESIDENT_WL) % 2 == 0:
                        wl_t = stream.tile([NPART, 2, ROWS], f16, tag="wl_t")
                        nc.gpsimd.dma_start(wl_t[:], wl[:, c : c + 2, :])
                        wl_rhs = wl_t[:, 0, :]
                    else:
                        wl_rhs = wl_t[:, 1, :]
                    for h in range(2):
                        sl = slice(h * HALF, (h + 1) * HALF)
                        nc.tensor.matmul(
                            acc[:, sl],
                            s_a[:, :, c],
                            wh_sb[:, c, sl],
                            start=(c == 0),
                            stop=False,
                        )
                        nc.tensor.matmul(
                            acc[:, sl],
                            s_b[:, :, c],
                            wl_rhs[:, sl],
                            start=False,
                            stop=(c == CHUNKS - 1),
                        )

                # tail: AllGather both PSUM rows (8KB), combine AFTER the
                # gather at [128, 64] shape where everything is cheap.
                u_sb = tmp.tile([2, ROWS], f32, tag="u_sb")
                nc.scalar.activation(
                    u_sb[:], acc[:], mybir.ActivationFunctionType.Copy
                )
                cc_in = dram.tile([2, ROWS], f32, name=f"cc_in_{it}")
                cc_out = dram.tile(
                    [N_CORES, 2, ROWS], f32, addr_space="Shared", name=f"cc_out_{it}"
                )
                nc.sync.dma_start(cc_in[:], u_sb[:])
                nc.gpsimd.collective_compute(
                    "AllGather",
                    mybir.AluOpType.bypass,
                    replica_groups=[list(range(N_CORES))],
                    ins=[cc_in[:]],
                    outs=[cc_out[:]],
                )
                # reload at [128, 2, 64] (k = p*64 + c): 8 per-rank DMAs,
                # triggers spread over both HWDGE engines to run in parallel
                u2 = tmp.tile([NPART, 2, CHUNKS], f32, tag="u2")
                q = NPART // N_CORES
                for r in range(N_CORES):
                    eng = nc.sync if r % 2 == 0 else nc.scalar
                    eng.dma_start(
                        u2[r * q : (r + 1) * q, :, :],
                        cc_out[r, :, :].rearrange("j (q c) -> q j c", c=CHUNKS),
                    )
                s_pre = tmp.tile([NPART, CHUNKS], f32, tag="s_pre")
                nc.vector.scalar_tensor_tensor(
                    s_pre[:],
                    u2[:, 1, :],
                    EPS,
                    u2[:, 0, :],
                    mybir.AluOpType.mult,
                    mybir.AluOpType.add,
                )
                s_f = tmp.tile([NPART, CHUNKS], f32, tag="s_f")
                nc.scalar.activation(
                    s_f[:], s_pre[:], mybir.ActivationFunctionType.Tanh
                )
                if it == ITERATIONS - 1:
                    # full final state, identical on every core
                    nc.sync.dma_start(out.rearrange("(p c) -> p c", p=NPART), s_f[:])
                else:
                    split_state(s_f)
                    # HAM warm-keepers: no-dependency matmuls into a junk
                    # PSUM bank, executed back-to-back during the gap.
                    junk = psum.tile([2, HALF], f32, tag="junk", bufs=1)
                    for _ in range(WARM_MMS):
                        nc.tensor.matmul(
                            junk[:],
                            wh_sb[:, 0, 0:2],
                            wh_sb[:, 0, 0:HALF],
                            start=True,
                            stop=True,
                        )
    nc.compile()
    return nc


def _prepare_in_maps(x, weights):
    x = np.ascontiguousarray(x, dtype=np.float32)
    w32 = np.asarray(weights, dtype=np.float32)
    in_maps = []
    for r in range(N_CORES):
        # k-major transpose of this core's row block: [8192 k, 1024 i]
        wt = np.ascontiguousarray(w32[r * ROWS : (r + 1) * ROWS, :].T)
        whi = wt.astype(np.float16)
        wlo = ((wt - whi.astype(np.float32)) * WL_SCALE).astype(np.float16)
        in_maps.append(
            {
                "xin": x,
                "wh": np.ascontiguousarray(whi.reshape(NPART, CHUNKS, ROWS)),
                "wl": np.ascontiguousarray(wlo.reshape(NPART, CHUNKS, ROWS)),
            }
        )
    return in_maps


def _run(inputs, **kwargs):
    if "nc" not in _CACHED:
        _CACHED["nc"] = _build_nc()
    nc = _CACHED["nc"]
    in_maps = _prepare_in_maps(inputs["x"], inputs["weights"])
    res = run_bass_kernel_spmd(nc, in_maps, core_ids=list(range(N_CORES)), **kwargs)
    out = res.results[0]["out"]
    return np.ascontiguousarray(out, dtype=np.float32), res


def kernel(**inputs) -> np.ndarray:
    out, _ = _run(inputs)
    return out
